# revision 9
# baseline (speedup 1.0000x reference)
"""Causal multi-head self-attention on 8 Trainium2 NeuronCores.

Problem: X[4, 2048, 1024] fp32, W_Q/W_K/W_V/W_O [1024, 1024] fp32,
16 heads x 64 dims, causal softmax attention + output projection.

Sharding: core c handles batch b = c//2 and head-group g = c%2
(heads g*8..g*8+8, i.e. 512 of the 1024 channels).  Each core computes
its 8 heads' Q/K/V projections, causal attention, and a partial output
projection against W_O[:, g*512:(g+1)*512]; the host sums the two
partial outputs per batch (the "all-reduce after W_O" step).

Device kernel layout notes:
 - All matmuls use float32r (fp32 bits, reduced-precision multiply) which
   streams at 1 cycle/row when the moving free dim >= 256, vs 4 cycles
   for full fp32.
 - Q/K are produced transposed ([channels, tokens]) so attention scores
   are computed as S_T[keys, q] = K_slice^T-free matmuls with Dh=64 on
   the contraction partitions; two heads (partition bases 0/64) run as
   row-tiled concurrent matmuls.
 - Softmax skips the max-subtraction (scores are bounded ~|1.9| after
   the 1/8 scale, applied via the activation's free affine).  exp runs
   on ScalarE over [128, 1024] PSUM groups.
 - Causal masking multiplies the diagonal score blocks by a 0/1 mask
   after exp (mask depends only on the key/query offset within the
   block, so one [128, 4, 512] constant serves every query tile).
 - V is stored [tokens, 512 ch + 64 ones]; using [V_head | ones] as the
   stationary operand of the P*V matmul makes PSUM rows 0..63 the
   unnormalized output and rows 64..127 the softmax row-sums, so the
   normalization is a reciprocal + multiply with no cross-partition
   reduction.
"""

import sys

if "/opt/trn_rl_repo" not in sys.path:
    sys.path.insert(0, "/opt/trn_rl_repo")

from contextlib import ExitStack

import numpy as np

import concourse.bacc as bacc
import concourse.bass as bass
import concourse.tile as tile
from concourse import mybir
from concourse.bass_utils import run_bass_kernel_spmd

B = 4
S = 2048
D = 1024
H = 16
DH = 64

P = 128
DIN_C = D // P        # 8 contraction chunks for the projections
CC = 4                # channel chunks per core (512 / 128)
NHEAD = 8             # heads per core
QT = S // 512         # query tiles of 512
TT = S // 512         # token tiles of 512
VH = 65               # per-head V block: 64 dims + 1 ones column

F32R = mybir.dt.float32r
F32 = mybir.dt.float32

LAST_RESULT = None
_NC_CACHE = None


def build_nc():
    nc = bacc.Bacc()

    xt_d = nc.dram_tensor("xt", [D, S], F32R, kind="ExternalInput")
    wqt_d = nc.dram_tensor("wqt", [D, 512], F32R, kind="ExternalInput")
    wkt_d = nc.dram_tensor("wkt", [D, 512], F32R, kind="ExternalInput")
    wvt_d = nc.dram_tensor("wvt", [D, 512], F32R, kind="ExternalInput")
    wot_d = nc.dram_tensor("wot", [512, D], F32R, kind="ExternalInput")
    mask_d = nc.dram_tensor("mask", [P, 4, 512], F32R, kind="ExternalInput")
    ones_d = nc.dram_tensor("ones", [P, (S // P) * NHEAD * VH], F32R, kind="ExternalInput")
    yt_d = nc.dram_tensor("yt", [D, S], F32, kind="ExternalOutput")

    xt_v = xt_d[:, :].rearrange("(kc p) t -> p kc t", p=P)
    wq_v = wqt_d[:, :].rearrange("(kc p) c -> p kc c", p=P)
    wk_v = wkt_d[:, :].rearrange("(kc p) c -> p kc c", p=P)
    wv_v = wvt_d[:, :].rearrange("(kc p) c -> p kc c", p=P)
    wot_v = wot_d[:, :].rearrange("(cc p) o -> p cc o", p=P)
    yt_v = yt_d[:, :]

    EXP = mybir.ActivationFunctionType.Exp

    with tile.TileContext(nc) as tc, ExitStack() as ctx:
        singles = ctx.enter_context(tc.tile_pool(name="singles", bufs=1))
        xt_pool = ctx.enter_context(tc.tile_pool(name="xtp", bufs=2))
        qk_pool = ctx.enter_context(tc.tile_pool(name="qkp", bufs=2))
        w_pool = ctx.enter_context(tc.tile_pool(name="wp", bufs=2))
        p_pool = ctx.enter_context(tc.tile_pool(name="pp", bufs=2))
        misc = ctx.enter_context(tc.tile_pool(name="misc", bufs=2))
        yt_pool = ctx.enter_context(tc.tile_pool(name="ytp", bufs=2))
        proj_ps = ctx.enter_context(tc.tile_pool(name="proj_ps", bufs=2, space="PSUM"))
        att_ps = ctx.enter_context(tc.tile_pool(name="att_ps", bufs=1, space="PSUM"))

        v_sb = singles.tile([P, S // P, NHEAD, VH], F32R)
        ones_sb = singles.tile([P, 64], F32R)
        ot_sb = singles.tile([P, CC, S], F32R)
        wot_sb = singles.tile([P, CC, D], F32R)
        mask_sb = singles.tile([P, 4, 512], F32R)
        nc.sync.dma_start(out=wot_sb, in_=wot_v)
        nc.sync.dma_start(out=mask_sb, in_=mask_d[:, :, :])
        nc.sync.dma_start(out=ones_sb, in_=ones_d[:, 0:64])
        # fill v_sb with 1.0 wholesale; V-projection copies overwrite the
        # data columns, leaving col 64 of each head block as the ones col.
        nc.sync.dma_start(out=v_sb[:, :, :, :], in_=ones_d[:, :])

        # ---- stage 0: V projection, tokens on partitions ----
        with tc.tile_pool(name="wvp", bufs=1) as wv_pool:
            wv_sb = wv_pool.tile([P, DIN_C, 512], F32R)
            nc.sync.dma_start(out=wv_sb, in_=wv_v)
            for tt in range(2 * TT):
                xt_t = xt_pool.tile([P, DIN_C, 256], F32R, tag="xt")
                nc.sync.dma_start(out=xt_t, in_=xt_v[:, :, tt * 256:(tt + 1) * 256])
                for sub in range(2):
                    vps = proj_ps.tile([P, 512], F32, tag="pp")
                    for kc in range(DIN_C):
                        nc.tensor.matmul(
                            vps,
                            xt_t[:, kc, sub * 128:(sub + 1) * 128],
                            wv_sb[:, kc, :],
                            start=(kc == 0),
                            stop=(kc == DIN_C - 1),
                        )
                    nc.vector.tensor_copy(v_sb[:, tt * 2 + sub, :, 0:64], vps)

        # ---- per channel-chunk: Q/K projection + attention for 2 heads ----
        for cc in range(CC):
            wq_sb = w_pool.tile([P, DIN_C, 128], F32R, tag="wq")
            wk_sb = w_pool.tile([P, DIN_C, 128], F32R, tag="wk")
            nc.sync.dma_start(out=wq_sb, in_=wq_v[:, :, cc * 128:(cc + 1) * 128])
            nc.sync.dma_start(out=wk_sb, in_=wk_v[:, :, cc * 128:(cc + 1) * 128])
            qt_sb = qk_pool.tile([P, S], F32R, tag="qt")
            kt_sb = qk_pool.tile([P, S], F32R, tag="kt")
            for tt in range(2 * TT):
                xt_t = xt_pool.tile([P, DIN_C, 256], F32R, tag="xt")
                nc.sync.dma_start(out=xt_t, in_=xt_v[:, :, tt * 256:(tt + 1) * 256])
                qps = proj_ps.tile([P, 256], F32, tag="pp", name=f"qps_{cc}_{tt}")
                for kc in range(DIN_C):
                    nc.tensor.matmul(
                        qps, wq_sb[:, kc, :], xt_t[:, kc, :],
                        start=(kc == 0), stop=(kc == DIN_C - 1),
                    )
                nc.vector.tensor_copy(qt_sb[:, tt * 256:(tt + 1) * 256], qps)
                kps = proj_ps.tile([P, 256], F32, tag="pp", name=f"kps_{cc}_{tt}")
                for kc in range(DIN_C):
                    nc.tensor.matmul(
                        kps, wk_sb[:, kc, :], xt_t[:, kc, :],
                        start=(kc == 0), stop=(kc == DIN_C - 1),
                    )
                nc.vector.tensor_copy(kt_sb[:, tt * 256:(tt + 1) * 256], kps)

            for qt in range(QT):
                ops = [att_ps.tile([P, 512], F32, tag=f"ops{h2}", name=f"ops{h2}_{cc}_{qt}") for h2 in range(2)]
                ngrp = 2 * qt + 2
                last_kc = 2 * ngrp - 1
                for grp in range(ngrp):
                    sps = [att_ps.tile([P, 1024], F32, tag=f"sps{h2}", name=f"sps{h2}_{cc}_{qt}_{grp}") for h2 in range(2)]
                    for j in range(2):
                        kc = grp * 2 + j
                        for h2 in range(2):
                            b0 = h2 * 64
                            nc.tensor.matmul(
                                sps[h2][:, j * 512:(j + 1) * 512],
                                kt_sb[b0:b0 + 64, kc * 128:(kc + 1) * 128],
                                qt_sb[b0:b0 + 64, qt * 512:(qt + 1) * 512],
                                start=True,
                                stop=True,
                            )
                    pts = []
                    for h2 in range(2):
                        p_t = p_pool.tile([P, 1024], F32R, tag=f"p{h2}", name=f"p{h2}_{cc}_{qt}_{grp}")
                        nc.scalar.activation(p_t, sps[h2], EXP, scale=0.125)
                        pts.append(p_t)
                    if grp >= 2 * qt:  # diagonal groups: causal mask
                        r0 = (grp - 2 * qt) * 2
                        for h2 in range(2):
                            nc.vector.tensor_mul(
                                pts[h2], pts[h2], mask_sb[:, r0:r0 + 2, :]
                            )
                    for j in range(2):
                        kc = grp * 2 + j
                        for h2 in range(2):
                            nc.tensor.matmul(
                                ops[h2][0:VH, :],
                                v_sb[:, kc, 2 * cc + h2, 0:VH],
                                pts[h2][:, j * 512:(j + 1) * 512],
                                start=(kc == 0),
                                stop=(kc == last_kc),
                            )
                for h2 in range(2):
                    # softmax sums live in ops row 64; reciprocal, broadcast
                    # across 64 partitions via a K=1 outer product on PE,
                    # then scale the unnormalized output rows.
                    rrow = misc.tile([P, 512], F32R, tag="rrow",
                                     name=f"rrow{h2}_{cc}_{qt}")
                    with nc.allow_low_precision(reason="float32r out is fp32-width"):
                        nc.vector.reciprocal(rrow[64:65, :], ops[h2][64:65, :])
                    rps = proj_ps.tile([64, 512], F32, tag="pp",
                                       name=f"rps{h2}_{cc}_{qt}")
                    nc.tensor.matmul(rps, ones_sb[64:65, :], rrow[64:65, :],
                                     start=True, stop=True)
                    rec = misc.tile([64, 512], F32, tag="rec",
                                    name=f"rec{h2}_{cc}_{qt}")
                    nc.vector.tensor_copy(rec, rps)
                    nc.vector.tensor_mul(
                        ot_sb[h2 * 64:h2 * 64 + 64, cc, qt * 512:(qt + 1) * 512],
                        ops[h2][0:64, :],
                        rec,
                    )

        # ---- output projection (partial, host sums across head groups) ----
        for oc in range(D // P):
            for tt in range(TT):
                ops_o = proj_ps.tile([P, 512], F32, tag="pp")
                for c2 in range(CC):
                    nc.tensor.matmul(
                        ops_o,
                        wot_sb[:, c2, oc * 128:(oc + 1) * 128],
                        ot_sb[:, c2, tt * 512:(tt + 1) * 512],
                        start=(c2 == 0),
                        stop=(c2 == CC - 1),
                    )
                y_t = yt_pool.tile([P, 512], F32, tag="yt")
                nc.vector.tensor_copy(y_t, ops_o)
                nc.sync.dma_start(
                    out=yt_v[oc * 128:(oc + 1) * 128, tt * 512:(tt + 1) * 512],
                    in_=y_t,
                )

    nc.finalize()
    return nc


def _make_mask():
    keys = np.arange(4)[None, :, None] * 128 + np.arange(128)[:, None, None]
    qs = np.arange(512)[None, None, :]
    return (keys <= qs).astype(np.float32)


def kernel(X, W_Q, W_K, W_V, W_O):
    global LAST_RESULT, _NC_CACHE
    X = np.asarray(X, dtype=np.float32)
    W_Q = np.asarray(W_Q, dtype=np.float32)
    W_K = np.asarray(W_K, dtype=np.float32)
    W_V = np.asarray(W_V, dtype=np.float32)
    W_O = np.asarray(W_O, dtype=np.float32)

    mask = _make_mask()
    in_maps = []
    for c in range(8):
        b, g = c // 2, c % 2
        sl = slice(g * 512, (g + 1) * 512)
        in_maps.append({
            "ones": np.ones((128, 16 * 8 * 65), dtype=np.float32),
            "xt": np.ascontiguousarray(X[b].T),
            "wqt": np.ascontiguousarray(W_Q[sl, :].T),
            "wkt": np.ascontiguousarray(W_K[sl, :].T),
            "wvt": np.ascontiguousarray(W_V[sl, :].T),
            "wot": np.ascontiguousarray(W_O[:, sl].T),
            "mask": mask,
        })

    if _NC_CACHE is None:
        _NC_CACHE = build_nc()
    res = run_bass_kernel_spmd(_NC_CACHE, in_maps, core_ids=list(range(8)))
    LAST_RESULT = res

    out = np.empty((B, S, D), dtype=np.float32)
    for b in range(B):
        yt = res.results[2 * b]["yt"] + res.results[2 * b + 1]["yt"]
        out[b] = yt.T
    return out


# revision 10
# speedup vs baseline: 1.1131x; 1.1131x over previous
"""Causal multi-head self-attention on 8 Trainium2 NeuronCores.

Problem: X[4, 2048, 1024] fp32, W_Q/W_K/W_V/W_O [1024, 1024] fp32,
16 heads x 64 dims, causal softmax attention + output projection.

Sharding: core c handles batch b = c//2 and head-group g = c%2
(heads g*8..g*8+8, i.e. 512 of the 1024 channels).  Each core computes
its 8 heads' Q/K/V projections, causal attention, and a partial output
projection against W_O[:, g*512:(g+1)*512]; the host sums the two
partial outputs per batch (the "all-reduce after W_O" step).

Device kernel layout notes:
 - All matmuls use float32r (fp32 bits, reduced-precision multiply) which
   streams at 1 cycle/row when the moving free dim >= 256, vs 4 cycles
   for full fp32.
 - Q/K are produced transposed ([channels, tokens]) so attention scores
   are computed as S_T[keys, q] = K_slice^T-free matmuls with Dh=64 on
   the contraction partitions; two heads (partition bases 0/64) run as
   row-tiled concurrent matmuls.
 - Softmax skips the max-subtraction (scores are bounded ~|1.9| after
   the 1/8 scale, applied via the activation's free affine).  exp runs
   on ScalarE over [128, 1024] PSUM groups.
 - Causal masking multiplies the diagonal score blocks by a 0/1 mask
   after exp (mask depends only on the key/query offset within the
   block, so one [128, 4, 512] constant serves every query tile).
 - V is stored [tokens, 512 ch + 64 ones]; using [V_head | ones] as the
   stationary operand of the P*V matmul makes PSUM rows 0..63 the
   unnormalized output and rows 64..127 the softmax row-sums, so the
   normalization is a reciprocal + multiply with no cross-partition
   reduction.
"""

import sys

if "/opt/trn_rl_repo" not in sys.path:
    sys.path.insert(0, "/opt/trn_rl_repo")

from contextlib import ExitStack

import ml_dtypes
import numpy as np

import concourse.bacc as bacc
import concourse.bass as bass
import concourse.tile as tile
from concourse import mybir
from concourse.bass_utils import run_bass_kernel_spmd

B = 4
S = 2048
D = 1024
H = 16
DH = 64

P = 128
DIN_C = D // P        # 8 contraction chunks for the projections
CC = 4                # channel chunks per core (512 / 128)
NHEAD = 8             # heads per core
QT = S // 512         # query tiles of 512
TT = S // 512         # token tiles of 512
VH = 65               # per-head V block: 64 dims + 1 ones column

F32R = mybir.dt.float32r
F32 = mybir.dt.float32
BF16 = mybir.dt.bfloat16

LAST_RESULT = None
_NC_CACHE = None


def build_nc():
    nc = bacc.Bacc()

    xt_d = nc.dram_tensor("xt", [D, S], F32R, kind="ExternalInput")
    wqt_d = nc.dram_tensor("wqt", [D, 512], F32R, kind="ExternalInput")
    wkt_d = nc.dram_tensor("wkt", [D, 512], F32R, kind="ExternalInput")
    wvt_d = nc.dram_tensor("wvt", [D, 512], F32R, kind="ExternalInput")
    wot_d = nc.dram_tensor("wot", [512, D], F32R, kind="ExternalInput")
    mask_d = nc.dram_tensor("mask", [P, 4, 512], BF16, kind="ExternalInput")
    ones_d = nc.dram_tensor("ones", [P, (S // P) * NHEAD * VH], BF16, kind="ExternalInput")
    yt_d = nc.dram_tensor("yt", [D, S], F32, kind="ExternalOutput")

    xt_v = xt_d[:, :].rearrange("(kc p) t -> p kc t", p=P)
    wq_v = wqt_d[:, :].rearrange("(kc p) c -> p kc c", p=P)
    wk_v = wkt_d[:, :].rearrange("(kc p) c -> p kc c", p=P)
    wv_v = wvt_d[:, :].rearrange("(kc p) c -> p kc c", p=P)
    wot_v = wot_d[:, :].rearrange("(cc p) o -> p cc o", p=P)
    yt_v = yt_d[:, :]

    EXP = mybir.ActivationFunctionType.Exp

    with tile.TileContext(nc) as tc, ExitStack() as ctx:
        singles = ctx.enter_context(tc.tile_pool(name="singles", bufs=1))
        xt_pool = ctx.enter_context(tc.tile_pool(name="xtp", bufs=2))
        qk_pool = ctx.enter_context(tc.tile_pool(name="qkp", bufs=2))
        w_pool = ctx.enter_context(tc.tile_pool(name="wp", bufs=1))
        p_pool = ctx.enter_context(tc.tile_pool(name="pp", bufs=2))
        misc = ctx.enter_context(tc.tile_pool(name="misc", bufs=2))
        yt_pool = ctx.enter_context(tc.tile_pool(name="ytp", bufs=2))
        proj_ps = ctx.enter_context(tc.tile_pool(name="proj_ps", bufs=2, space="PSUM"))
        att_ps = ctx.enter_context(tc.tile_pool(name="att_ps", bufs=1, space="PSUM"))

        v_sb = singles.tile([P, S // P, NHEAD, VH], BF16)
        ones_sb = singles.tile([P, 64], F32R)
        ot_sb = singles.tile([P, CC, S], F32R)
        wot_sb = singles.tile([P, CC, D], F32R)
        mask_sb = singles.tile([P, 4, 512], BF16)
        nc.sync.dma_start(out=wot_sb, in_=wot_v)
        nc.sync.dma_start(out=mask_sb, in_=mask_d[:, :, :])
        nc.gpsimd.dma_start(out=ones_sb, in_=ones_d[:, 0:64])
        # fill v_sb with 1.0 wholesale; V-projection copies overwrite the
        # data columns, leaving col 64 of each head block as the ones col.
        nc.sync.dma_start(out=v_sb[:, :, :, :], in_=ones_d[:, :])

        # ---- stage 0: V projection, tokens on partitions ----
        with tc.tile_pool(name="wvp", bufs=1) as wv_pool:
            wv_sb = wv_pool.tile([P, DIN_C, 512], F32R)
            nc.sync.dma_start(out=wv_sb, in_=wv_v)
            for tt in range(TT):
                xt_t = xt_pool.tile([P, DIN_C, 512], F32R, tag="xt")
                nc.sync.dma_start(out=xt_t, in_=xt_v[:, :, tt * 512:(tt + 1) * 512])
                for sub in range(4):
                    vps = proj_ps.tile([P, 512], F32, tag="pp")
                    for kc in range(DIN_C):
                        nc.tensor.matmul(
                            vps,
                            xt_t[:, kc, sub * 128:(sub + 1) * 128],
                            wv_sb[:, kc, :],
                            start=(kc == 0),
                            stop=(kc == DIN_C - 1),
                        )
                    nc.vector.tensor_copy(v_sb[:, tt * 4 + sub, :, 0:64], vps)

        # ---- per channel-chunk: Q/K projection + attention for 2 heads ----
        for cc in range(CC):
            wq_sb = w_pool.tile([P, DIN_C, 128], F32R, tag="wq")
            wk_sb = w_pool.tile([P, DIN_C, 128], F32R, tag="wk")
            nc.sync.dma_start(out=wq_sb, in_=wq_v[:, :, cc * 128:(cc + 1) * 128])
            nc.sync.dma_start(out=wk_sb, in_=wk_v[:, :, cc * 128:(cc + 1) * 128])
            qt_sb = qk_pool.tile([P, S], F32R, tag="qt")
            kt_sb = qk_pool.tile([P, S], F32R, tag="kt")
            for tt in range(TT):
                xt_t = xt_pool.tile([P, DIN_C, 512], F32R, tag="xt")
                nc.sync.dma_start(out=xt_t, in_=xt_v[:, :, tt * 512:(tt + 1) * 512])
                qps = proj_ps.tile([P, 512], F32, tag="pp", name=f"qps_{cc}_{tt}")
                for kc in range(DIN_C):
                    nc.tensor.matmul(
                        qps, wq_sb[:, kc, :], xt_t[:, kc, :],
                        start=(kc == 0), stop=(kc == DIN_C - 1),
                    )
                nc.vector.tensor_copy(qt_sb[:, tt * 512:(tt + 1) * 512], qps)
                kps = proj_ps.tile([P, 512], F32, tag="pp", name=f"kps_{cc}_{tt}")
                for kc in range(DIN_C):
                    nc.tensor.matmul(
                        kps, wk_sb[:, kc, :], xt_t[:, kc, :],
                        start=(kc == 0), stop=(kc == DIN_C - 1),
                    )
                nc.vector.tensor_copy(kt_sb[:, tt * 512:(tt + 1) * 512], kps)

            for qt in range(QT):
                ops = [att_ps.tile([P, 512], F32, tag=f"ops{h2}", name=f"ops{h2}_{cc}_{qt}") for h2 in range(2)]
                ngrp = 2 * qt + 2
                last_kc = 2 * ngrp - 1
                for grp in range(ngrp):
                    sps = [att_ps.tile([P, 1024], F32, tag=f"sps{h2}", name=f"sps{h2}_{cc}_{qt}_{grp}") for h2 in range(2)]
                    for j in range(2):
                        kc = grp * 2 + j
                        for h2 in range(2):
                            b0 = h2 * 64
                            nc.tensor.matmul(
                                sps[h2][:, j * 512:(j + 1) * 512],
                                kt_sb[b0:b0 + 64, kc * 128:(kc + 1) * 128],
                                qt_sb[b0:b0 + 64, qt * 512:(qt + 1) * 512],
                                start=True,
                                stop=True,
                            )
                    pts = []
                    for h2 in range(2):
                        p_t = p_pool.tile([P, 1024], BF16, tag=f"p{h2}", name=f"p{h2}_{cc}_{qt}_{grp}")
                        nc.scalar.activation(p_t, sps[h2], EXP, scale=0.125)
                        pts.append(p_t)
                    if grp >= 2 * qt:  # diagonal groups: causal mask
                        r0 = (grp - 2 * qt) * 2
                        for h2 in range(2):
                            nc.vector.tensor_mul(
                                pts[h2], pts[h2], mask_sb[:, r0:r0 + 2, :]
                            )
                    for j in range(2):
                        kc = grp * 2 + j
                        for h2 in range(2):
                            nc.tensor.matmul(
                                ops[h2][0:VH, :],
                                v_sb[:, kc, 2 * cc + h2, 0:VH],
                                pts[h2][:, j * 512:(j + 1) * 512],
                                start=(kc == 0),
                                stop=(kc == last_kc),
                            )
                for h2 in range(2):
                    # softmax sums live in ops row 64; reciprocal, broadcast
                    # across 64 partitions via a K=1 outer product on PE,
                    # then scale the unnormalized output rows.
                    rrow = misc.tile([P, 512], F32R, tag="rrow",
                                     name=f"rrow{h2}_{cc}_{qt}")
                    with nc.allow_low_precision(reason="float32r out is fp32-width"):
                        nc.vector.reciprocal(rrow[64:65, :], ops[h2][64:65, :])
                    rps = proj_ps.tile([64, 512], F32, tag="pp",
                                       name=f"rps{h2}_{cc}_{qt}")
                    nc.tensor.matmul(rps, ones_sb[64:65, :], rrow[64:65, :],
                                     start=True, stop=True)
                    rec = misc.tile([64, 512], F32, tag="rec",
                                    name=f"rec{h2}_{cc}_{qt}")
                    nc.vector.tensor_copy(rec, rps)
                    nc.vector.tensor_mul(
                        ot_sb[h2 * 64:h2 * 64 + 64, cc, qt * 512:(qt + 1) * 512],
                        ops[h2][0:64, :],
                        rec,
                    )

        # ---- output projection (partial, host sums across head groups) ----
        for oc in range(D // P):
            for tt in range(TT):
                ops_o = proj_ps.tile([P, 512], F32, tag="pp")
                for c2 in range(CC):
                    nc.tensor.matmul(
                        ops_o,
                        wot_sb[:, c2, oc * 128:(oc + 1) * 128],
                        ot_sb[:, c2, tt * 512:(tt + 1) * 512],
                        start=(c2 == 0),
                        stop=(c2 == CC - 1),
                    )
                y_t = yt_pool.tile([P, 512], F32, tag="yt")
                nc.vector.tensor_copy(y_t, ops_o)
                nc.sync.dma_start(
                    out=yt_v[oc * 128:(oc + 1) * 128, tt * 512:(tt + 1) * 512],
                    in_=y_t,
                )

    nc.finalize()
    return nc


def _make_mask():
    keys = np.arange(4)[None, :, None] * 128 + np.arange(128)[:, None, None]
    qs = np.arange(512)[None, None, :]
    return (keys <= qs).astype(np.float32)


def kernel(X, W_Q, W_K, W_V, W_O):
    global LAST_RESULT, _NC_CACHE
    X = np.asarray(X, dtype=np.float32)
    W_Q = np.asarray(W_Q, dtype=np.float32)
    W_K = np.asarray(W_K, dtype=np.float32)
    W_V = np.asarray(W_V, dtype=np.float32)
    W_O = np.asarray(W_O, dtype=np.float32)

    mask = _make_mask().astype(ml_dtypes.bfloat16)
    in_maps = []
    for c in range(8):
        b, g = c // 2, c % 2
        sl = slice(g * 512, (g + 1) * 512)
        in_maps.append({
            "ones": np.ones((128, 16 * 8 * 65), dtype=ml_dtypes.bfloat16),
            "xt": np.ascontiguousarray(X[b].T),
            "wqt": np.ascontiguousarray(W_Q[sl, :].T),
            "wkt": np.ascontiguousarray(W_K[sl, :].T),
            "wvt": np.ascontiguousarray(W_V[sl, :].T),
            "wot": np.ascontiguousarray(W_O[:, sl].T),
            "mask": mask,
        })

    if _NC_CACHE is None:
        _NC_CACHE = build_nc()
    res = run_bass_kernel_spmd(_NC_CACHE, in_maps, core_ids=list(range(8)))
    LAST_RESULT = res

    out = np.empty((B, S, D), dtype=np.float32)
    for b in range(B):
        yt = res.results[2 * b]["yt"] + res.results[2 * b + 1]["yt"]
        out[b] = yt.T
    return out


# revision 12
# speedup vs baseline: 1.2147x; 1.0913x over previous
"""Causal multi-head self-attention on 8 Trainium2 NeuronCores.

Problem: X[4, 2048, 1024] fp32, W_Q/W_K/W_V/W_O [1024, 1024] fp32,
16 heads x 64 dims, causal softmax attention + output projection.

Sharding: core c handles batch b = c//2 and head-group g = c%2
(heads g*8..g*8+8, i.e. 512 of the 1024 channels).  Each core computes
its 8 heads' Q/K/V projections, causal attention, and a partial output
projection against W_O[:, g*512:(g+1)*512]; the host sums the two
partial outputs per batch (the "all-reduce after W_O" step).

Device kernel layout notes:
 - All matmuls use float32r (fp32 bits, reduced-precision multiply) which
   streams at 1 cycle/row when the moving free dim >= 256, vs 4 cycles
   for full fp32.
 - Q/K are produced transposed ([channels, tokens]) so attention scores
   are computed as S_T[keys, q] = K_slice^T-free matmuls with Dh=64 on
   the contraction partitions; two heads (partition bases 0/64) run as
   row-tiled concurrent matmuls.
 - Softmax skips the max-subtraction (scores are bounded ~|1.9| after
   the 1/8 scale, applied via the activation's free affine).  exp runs
   on ScalarE over [128, 1024] PSUM groups.
 - Causal masking multiplies the diagonal score blocks by a 0/1 mask
   after exp (mask depends only on the key/query offset within the
   block, so one [128, 4, 512] constant serves every query tile).
 - V is stored [tokens, 512 ch + 64 ones]; using [V_head | ones] as the
   stationary operand of the P*V matmul makes PSUM rows 0..63 the
   unnormalized output and rows 64..127 the softmax row-sums, so the
   normalization is a reciprocal + multiply with no cross-partition
   reduction.
"""

import sys

if "/opt/trn_rl_repo" not in sys.path:
    sys.path.insert(0, "/opt/trn_rl_repo")

from contextlib import ExitStack

import ml_dtypes
import numpy as np

import concourse.bacc as bacc
import concourse.bass as bass
import concourse.hw_specs as _hw_specs
import concourse.tile as tile
from concourse import mybir
from concourse.bass_utils import run_bass_kernel_spmd

# Bias the activation-table chooser so Exp resolves to the set that also
# contains Ln ("natural_log_exp_and_others"): the kernel interleaves Exp
# (softmax) with Ln (reciprocal via exp(-ln s)), and per-function minimal
# sets would thrash the ~2.7us ACT table load on every switch.
_orig_get_activation_tables = _hw_specs.get_activation_tables


def _patched_activation_tables(arch):
    exp_fn = mybir.ActivationFunctionType.Exp
    out = {}
    for name, fns in _orig_get_activation_tables(arch).items():
        if name != "natural_log_exp_and_others" and exp_fn in fns:
            fns = [f for f in fns if f != exp_fn]
        out[name] = set(fns)
    return out


bacc.get_activation_tables = _patched_activation_tables

B = 4
S = 2048
D = 1024
H = 16
DH = 64

P = 128
DIN_C = D // P        # 8 contraction chunks for the projections
CC = 4                # channel chunks per core (512 / 128)
NHEAD = 8             # heads per core
QT = S // 512         # query tiles of 512
TT = S // 512         # token tiles of 512
VH = 65               # per-head V block: 64 dims + 1 ones column

F32R = mybir.dt.float32r
F32 = mybir.dt.float32
BF16 = mybir.dt.bfloat16

LAST_RESULT = None
_NC_CACHE = None


def build_nc():
    nc = bacc.Bacc()

    xt_d = nc.dram_tensor("xt", [D, S], BF16, kind="ExternalInput")
    wqt_d = nc.dram_tensor("wqt", [D, 512], BF16, kind="ExternalInput")
    wkt_d = nc.dram_tensor("wkt", [D, 512], BF16, kind="ExternalInput")
    wvt_d = nc.dram_tensor("wvt", [D, 512], BF16, kind="ExternalInput")
    wot_d = nc.dram_tensor("wot", [512, D], BF16, kind="ExternalInput")
    mask_d = nc.dram_tensor("mask", [P, 4, 512], BF16, kind="ExternalInput")
    ones_d = nc.dram_tensor("ones", [P, (S // P) * NHEAD * VH], BF16, kind="ExternalInput")
    yt_d = nc.dram_tensor("yt", [D, S], F32, kind="ExternalOutput")

    xt_v = xt_d[:, :].rearrange("(kc p) t -> p kc t", p=P)
    wq_v = wqt_d[:, :].rearrange("(kc p) c -> p kc c", p=P)
    wk_v = wkt_d[:, :].rearrange("(kc p) c -> p kc c", p=P)
    wv_v = wvt_d[:, :].rearrange("(kc p) c -> p kc c", p=P)
    wot_v = wot_d[:, :].rearrange("(cc p) o -> p cc o", p=P)
    yt_v = yt_d[:, :]

    EXP = mybir.ActivationFunctionType.Exp

    with tile.TileContext(nc) as tc, ExitStack() as ctx:
        singles = ctx.enter_context(tc.tile_pool(name="singles", bufs=1))
        xt_pool = ctx.enter_context(tc.tile_pool(name="xtp", bufs=2))
        qk_pool = ctx.enter_context(tc.tile_pool(name="qkp", bufs=2))
        w_pool = ctx.enter_context(tc.tile_pool(name="wp", bufs=1))
        p_pool = ctx.enter_context(tc.tile_pool(name="pp", bufs=2))
        misc = ctx.enter_context(tc.tile_pool(name="misc", bufs=2))
        yt_pool = ctx.enter_context(tc.tile_pool(name="ytp", bufs=2))
        proj_ps = ctx.enter_context(tc.tile_pool(name="proj_ps", bufs=2, space="PSUM"))
        att_ps = ctx.enter_context(tc.tile_pool(name="att_ps", bufs=1, space="PSUM"))

        v_sb = singles.tile([P, S // P, NHEAD, VH], BF16)
        ones_sb = singles.tile([P, 64], F32R)
        ot_sb = singles.tile([P, CC, S], BF16)
        wot_sb = singles.tile([P, CC, D], BF16)
        mask_sb = singles.tile([P, 4, 512], BF16)
        # fill v_sb with 1.0 wholesale; V-projection copies overwrite the
        # data columns, leaving col 64 of each head block as the ones col.
        nc.sync.dma_start(out=v_sb[:, :, :, :], in_=ones_d[:, :])
        nc.gpsimd.dma_start(out=ones_sb, in_=ones_d[:, 0:64])
        nc.sync.dma_start(out=mask_sb, in_=mask_d[:, :, :])
        nc.sync.dma_start(out=wot_sb, in_=wot_v)

        # ---- stage 0: V projection, tokens on partitions ----
        with tc.tile_pool(name="wvp", bufs=1) as wv_pool:
            wv_sb = wv_pool.tile([P, DIN_C, 512], BF16)
            nc.sync.dma_start(out=wv_sb, in_=wv_v)
            for tt in range(TT):
                xt_t = xt_pool.tile([P, DIN_C, 512], BF16, tag="xt")
                nc.sync.dma_start(out=xt_t[:, 0:4, :],
                                  in_=xt_v[:, 0:4, tt * 512:(tt + 1) * 512])
                nc.sync.dma_start(out=xt_t[:, 4:8, :],
                                  in_=xt_v[:, 4:8, tt * 512:(tt + 1) * 512])
                for sub in range(4):
                    vps = proj_ps.tile([P, 512], F32, tag="pp")
                    for kc in range(DIN_C):
                        nc.tensor.matmul(
                            vps,
                            xt_t[:, kc, sub * 128:(sub + 1) * 128],
                            wv_sb[:, kc, :],
                            start=(kc == 0),
                            stop=(kc == DIN_C - 1),
                        )
                    nc.vector.tensor_copy(v_sb[:, tt * 4 + sub, :, 0:64], vps)

        # ---- per channel-chunk: Q/K projection + attention for 2 heads ----
        for cc in range(CC):
            wq_sb = w_pool.tile([P, DIN_C, 128], BF16, tag="wq")
            wk_sb = w_pool.tile([P, DIN_C, 128], BF16, tag="wk")
            nc.sync.dma_start(out=wq_sb, in_=wq_v[:, :, cc * 128:(cc + 1) * 128])
            nc.sync.dma_start(out=wk_sb, in_=wk_v[:, :, cc * 128:(cc + 1) * 128])
            qt_sb = qk_pool.tile([P, S], BF16, tag="qt")
            kt_sb = qk_pool.tile([P, S], BF16, tag="kt")
            for tt in range(TT):
                xt_t = xt_pool.tile([P, DIN_C, 512], BF16, tag="xt")
                nc.sync.dma_start(out=xt_t[:, 0:4, :],
                                  in_=xt_v[:, 0:4, tt * 512:(tt + 1) * 512])
                nc.sync.dma_start(out=xt_t[:, 4:8, :],
                                  in_=xt_v[:, 4:8, tt * 512:(tt + 1) * 512])
                qps = proj_ps.tile([P, 512], F32, tag="pp", name=f"qps_{cc}_{tt}")
                for kc in range(DIN_C):
                    nc.tensor.matmul(
                        qps, wq_sb[:, kc, :], xt_t[:, kc, :],
                        start=(kc == 0), stop=(kc == DIN_C - 1),
                    )
                nc.vector.tensor_copy(qt_sb[:, tt * 512:(tt + 1) * 512], qps)
                kps = proj_ps.tile([P, 512], F32, tag="pp", name=f"kps_{cc}_{tt}")
                for kc in range(DIN_C):
                    nc.tensor.matmul(
                        kps, wk_sb[:, kc, :], xt_t[:, kc, :],
                        start=(kc == 0), stop=(kc == DIN_C - 1),
                    )
                nc.vector.tensor_copy(kt_sb[:, tt * 512:(tt + 1) * 512], kps)

            for qt in range(QT):
                ops = [att_ps.tile([P, 512], F32, tag=f"ops{h2}", name=f"ops{h2}_{cc}_{qt}") for h2 in range(2)]
                ngrp = 2 * qt + 2
                last_kc = 2 * ngrp - 1
                for grp in range(ngrp):
                    sps = [att_ps.tile([P, 1024], F32, tag=f"sps{h2}", name=f"sps{h2}_{cc}_{qt}_{grp}") for h2 in range(2)]
                    for j in range(2):
                        kc = grp * 2 + j
                        for h2 in range(2):
                            b0 = h2 * 64
                            nc.tensor.matmul(
                                sps[h2][:, j * 512:(j + 1) * 512],
                                kt_sb[b0:b0 + 64, kc * 128:(kc + 1) * 128],
                                qt_sb[b0:b0 + 64, qt * 512:(qt + 1) * 512],
                                start=True,
                                stop=True,
                            )
                    pts = []
                    for h2 in range(2):
                        p_t = p_pool.tile([P, 1024], BF16, tag=f"p{h2}", name=f"p{h2}_{cc}_{qt}_{grp}")
                        nc.scalar.activation(p_t, sps[h2], EXP, scale=0.125)
                        pts.append(p_t)
                    if grp >= 2 * qt:  # diagonal groups: causal mask
                        r0 = (grp - 2 * qt) * 2
                        for h2 in range(2):
                            for j in range(2):
                                nc.vector.tensor_mul(
                                    pts[h2][:, j * 512:(j + 1) * 512],
                                    pts[h2][:, j * 512:(j + 1) * 512],
                                    mask_sb[:, r0 + j, :],
                                )
                    for j in range(2):
                        kc = grp * 2 + j
                        for h2 in range(2):
                            nc.tensor.matmul(
                                ops[h2][0:VH, :],
                                v_sb[:, kc, 2 * cc + h2, 0:VH],
                                pts[h2][:, j * 512:(j + 1) * 512],
                                start=(kc == 0),
                                stop=(kc == last_kc),
                            )
                for h2 in range(2):
                    # Move U out of PSUM immediately (frees the opsum bank
                    # for the next query tile), compute 1/s = exp(-ln s) on
                    # ScalarE (DVE's iterative divide is ~8 cyc/elem on one
                    # lane), broadcast it across 64 partitions via a K=1
                    # outer product on PE, then scale U.
                    u_sb = misc.tile([64, 512], F32, tag=f"u{h2}",
                                     name=f"u{h2}_{cc}_{qt}")
                    nc.vector.tensor_copy(u_sb, ops[h2][0:64, :])
                    rrow = misc.tile([P, 512], F32R, tag="rrow",
                                     name=f"rrow{h2}_{cc}_{qt}")
                    nc.scalar.activation(rrow[64:65, :], ops[h2][64:65, :],
                                         mybir.ActivationFunctionType.Ln)
                    rexp = misc.tile([P, 512], F32R, tag="rexp",
                                     name=f"rexp{h2}_{cc}_{qt}")
                    nc.scalar.activation(rexp[64:65, :], rrow[64:65, :],
                                         EXP, scale=-1.0)
                    rps = proj_ps.tile([64, 512], F32, tag="pp",
                                       name=f"rps{h2}_{cc}_{qt}")
                    nc.tensor.matmul(rps, ones_sb[64:65, :], rexp[64:65, :],
                                     start=True, stop=True)
                    rec = misc.tile([64, 512], F32, tag="rec",
                                    name=f"rec{h2}_{cc}_{qt}")
                    nc.vector.tensor_copy(rec, rps)
                    nc.vector.tensor_mul(
                        ot_sb[h2 * 64:h2 * 64 + 64, cc, qt * 512:(qt + 1) * 512],
                        u_sb,
                        rec,
                    )

        # ---- output projection (partial, host sums across head groups) ----
        for oc in range(D // P):
            for tt in range(TT):
                ops_o = proj_ps.tile([P, 512], F32, tag="pp")
                for c2 in range(CC):
                    nc.tensor.matmul(
                        ops_o,
                        wot_sb[:, c2, oc * 128:(oc + 1) * 128],
                        ot_sb[:, c2, tt * 512:(tt + 1) * 512],
                        start=(c2 == 0),
                        stop=(c2 == CC - 1),
                    )
                y_t = yt_pool.tile([P, 512], F32, tag="yt")
                nc.vector.tensor_copy(y_t, ops_o)
                nc.sync.dma_start(
                    out=yt_v[oc * 128:(oc + 1) * 128, tt * 512:(tt + 1) * 512],
                    in_=y_t,
                )

    nc.finalize()
    return nc


def _make_mask():
    keys = np.arange(4)[None, :, None] * 128 + np.arange(128)[:, None, None]
    qs = np.arange(512)[None, None, :]
    return (keys <= qs).astype(np.float32)


def kernel(X, W_Q, W_K, W_V, W_O):
    global LAST_RESULT, _NC_CACHE
    X = np.asarray(X, dtype=np.float32)
    W_Q = np.asarray(W_Q, dtype=np.float32)
    W_K = np.asarray(W_K, dtype=np.float32)
    W_V = np.asarray(W_V, dtype=np.float32)
    W_O = np.asarray(W_O, dtype=np.float32)

    mask = _make_mask().astype(ml_dtypes.bfloat16)
    in_maps = []
    for c in range(8):
        b, g = c // 2, c % 2
        sl = slice(g * 512, (g + 1) * 512)
        in_maps.append({
            "ones": np.ones((128, 16 * 8 * 65), dtype=ml_dtypes.bfloat16),
            "xt": np.ascontiguousarray(X[b].T).astype(ml_dtypes.bfloat16),
            "wqt": np.ascontiguousarray(W_Q[sl, :].T).astype(ml_dtypes.bfloat16),
            "wkt": np.ascontiguousarray(W_K[sl, :].T).astype(ml_dtypes.bfloat16),
            "wvt": np.ascontiguousarray(W_V[sl, :].T).astype(ml_dtypes.bfloat16),
            "wot": np.ascontiguousarray(W_O[:, sl].T).astype(ml_dtypes.bfloat16),
            "mask": mask,
        })

    if _NC_CACHE is None:
        _NC_CACHE = build_nc()
    res = run_bass_kernel_spmd(_NC_CACHE, in_maps, core_ids=list(range(8)))
    LAST_RESULT = res

    out = np.empty((B, S, D), dtype=np.float32)
    for b in range(B):
        yt = res.results[2 * b]["yt"] + res.results[2 * b + 1]["yt"]
        out[b] = yt.T
    return out


# revision 14
# speedup vs baseline: 1.5097x; 1.2428x over previous
"""Causal multi-head self-attention on 8 Trainium2 NeuronCores.

Problem: X[4, 2048, 1024] fp32, W_Q/W_K/W_V/W_O [1024, 1024] fp32,
16 heads x 64 dims, causal softmax attention + output projection.

Sharding: core c handles batch b = c//2 and head-group g = c%2
(heads g*8..g*8+8, i.e. 512 of the 1024 channels).  Each core computes
its 8 heads' Q/K/V projections, causal attention, and a partial output
projection against W_O[:, g*512:(g+1)*512]; the host sums the two
partial outputs per batch (the "all-reduce after W_O" step).

Device kernel layout notes:
 - All matmuls use float32r (fp32 bits, reduced-precision multiply) which
   streams at 1 cycle/row when the moving free dim >= 256, vs 4 cycles
   for full fp32.
 - Q/K are produced transposed ([channels, tokens]) so attention scores
   are computed as S_T[keys, q] = K_slice^T-free matmuls with Dh=64 on
   the contraction partitions; two heads (partition bases 0/64) run as
   row-tiled concurrent matmuls.
 - Softmax skips the max-subtraction (scores are bounded ~|1.9| after
   the 1/8 scale, applied via the activation's free affine).  exp runs
   on ScalarE over [128, 1024] PSUM groups.
 - Causal masking multiplies the diagonal score blocks by a 0/1 mask
   after exp (mask depends only on the key/query offset within the
   block, so one [128, 4, 512] constant serves every query tile).
 - V is stored [tokens, 512 ch + 64 ones]; using [V_head | ones] as the
   stationary operand of the P*V matmul makes PSUM rows 0..63 the
   unnormalized output and rows 64..127 the softmax row-sums, so the
   normalization is a reciprocal + multiply with no cross-partition
   reduction.
"""

import sys

if "/opt/trn_rl_repo" not in sys.path:
    sys.path.insert(0, "/opt/trn_rl_repo")

from contextlib import ExitStack

import ml_dtypes
import numpy as np

import concourse.bacc as bacc
import concourse.bass as bass
import concourse.hw_specs as _hw_specs
import concourse.tile as tile
from concourse import mybir
from concourse.bass_utils import run_bass_kernel_spmd

# Bias the activation-table chooser so Exp resolves to the set that also
# contains Ln ("natural_log_exp_and_others"): the kernel interleaves Exp
# (softmax) with Ln (reciprocal via exp(-ln s)), and per-function minimal
# sets would thrash the ~2.7us ACT table load on every switch.
_orig_get_activation_tables = _hw_specs.get_activation_tables


def _patched_activation_tables(arch):
    exp_fn = mybir.ActivationFunctionType.Exp
    out = {}
    for name, fns in _orig_get_activation_tables(arch).items():
        if name != "natural_log_exp_and_others" and exp_fn in fns:
            fns = [f for f in fns if f != exp_fn]
        out[name] = set(fns)
    return out


bacc.get_activation_tables = _patched_activation_tables

B = 4
S = 2048
D = 1024
H = 16
DH = 64

P = 128
DIN_C = D // P        # 8 contraction chunks for the projections
CC = 4                # channel chunks per core (512 / 128)
NHEAD = 8             # heads per core
QT = S // 512         # query tiles of 512
TT = S // 512         # token tiles of 512
VH = 65               # per-head V block: 64 dims + 1 ones column

F32R = mybir.dt.float32r
F32 = mybir.dt.float32
BF16 = mybir.dt.bfloat16

LAST_RESULT = None
_NC_CACHE = None


def build_nc():
    nc = bacc.Bacc()

    xt_d = nc.dram_tensor("xt", [D, S], BF16, kind="ExternalInput")
    wqt_d = nc.dram_tensor("wqt", [D, 512], BF16, kind="ExternalInput")
    wkt_d = nc.dram_tensor("wkt", [D, 512], BF16, kind="ExternalInput")
    wvt_d = nc.dram_tensor("wvt", [D, 512], BF16, kind="ExternalInput")
    wot_d = nc.dram_tensor("wot", [512, D], BF16, kind="ExternalInput")
    mask_d = nc.dram_tensor("mask", [P, 4, 512], BF16, kind="ExternalInput")
    ones_d = nc.dram_tensor("ones", [P, (S // P) * NHEAD * VH], BF16, kind="ExternalInput")
    yt_d = nc.dram_tensor("yt", [D, S], F32, kind="ExternalOutput")

    xt_v = xt_d[:, :].rearrange("(kc p) t -> p kc t", p=P)
    wq_v = wqt_d[:, :].rearrange("(kc p) c -> p kc c", p=P)
    wk_v = wkt_d[:, :].rearrange("(kc p) c -> p kc c", p=P)
    wv_v = wvt_d[:, :].rearrange("(kc p) c -> p kc c", p=P)
    wot_v = wot_d[:, :].rearrange("(cc p) o -> p cc o", p=P)
    yt_v = yt_d[:, :]

    EXP = mybir.ActivationFunctionType.Exp

    with tile.TileContext(nc) as tc, ExitStack() as ctx:
        singles = ctx.enter_context(tc.tile_pool(name="singles", bufs=1))
        xt_pool = ctx.enter_context(tc.tile_pool(name="xtp", bufs=2))
        qk_pool = ctx.enter_context(tc.tile_pool(name="qkp", bufs=2))
        w_pool = ctx.enter_context(tc.tile_pool(name="wp", bufs=1))
        p_pool = ctx.enter_context(tc.tile_pool(name="pp", bufs=2))
        misc = ctx.enter_context(tc.tile_pool(name="misc", bufs=2))
        yt_pool = ctx.enter_context(tc.tile_pool(name="ytp", bufs=2))
        proj_ps = ctx.enter_context(tc.tile_pool(name="proj_ps", bufs=2, space="PSUM"))
        dram_pool = ctx.enter_context(tc.tile_pool(name="drp", bufs=2, space="DRAM"))
        att_ps = ctx.enter_context(tc.tile_pool(name="att_ps", bufs=1, space="PSUM"))

        v_sb = singles.tile([P, S // P, NHEAD, VH], BF16)
        ot_sb = singles.tile([P, CC, S], BF16)
        wot_sb = singles.tile([P, CC, D], BF16)
        mask_sb = singles.tile([P, 4, 512], BF16)
        # fill v_sb with 1.0 wholesale; V-projection copies overwrite the
        # data columns, leaving col 64 of each head block as the ones col.
        nc.sync.dma_start(out=v_sb[:, :, :, :], in_=ones_d[:, :])
        nc.sync.dma_start(out=mask_sb, in_=mask_d[:, :, :])
        nc.sync.dma_start(out=wot_sb, in_=wot_v)

        # ---- stage 0: V projection, tokens on partitions ----
        with tc.tile_pool(name="wvp", bufs=1) as wv_pool:
            wv_sb = wv_pool.tile([P, DIN_C, 512], BF16)
            nc.sync.dma_start(out=wv_sb[:, 0:2, :], in_=wv_v[:, 0:2, :])
            nc.sync.dma_start(out=wv_sb[:, 2:8, :], in_=wv_v[:, 2:8, :])
            for tt in range(TT):
                xt_t = xt_pool.tile([P, DIN_C, 512], BF16, tag="xt")
                nc.sync.dma_start(out=xt_t[:, 0:4, :],
                                  in_=xt_v[:, 0:4, tt * 512:(tt + 1) * 512])
                nc.sync.dma_start(out=xt_t[:, 4:8, :],
                                  in_=xt_v[:, 4:8, tt * 512:(tt + 1) * 512])
                for sub in range(4):
                    vps = proj_ps.tile([P, 512], F32, tag="pp")
                    for kc in range(DIN_C):
                        nc.tensor.matmul(
                            vps,
                            xt_t[:, kc, sub * 128:(sub + 1) * 128],
                            wv_sb[:, kc, :],
                            start=(kc == 0),
                            stop=(kc == DIN_C - 1),
                        )
                    nc.vector.tensor_copy(v_sb[:, tt * 4 + sub, :, 0:64], vps)

        # ---- per channel-chunk: Q/K projection + attention for 2 heads ----
        for cc in range(CC):
            wq_sb = w_pool.tile([P, DIN_C, 128], BF16, tag="wq")
            wk_sb = w_pool.tile([P, DIN_C, 128], BF16, tag="wk")
            nc.sync.dma_start(out=wq_sb, in_=wq_v[:, :, cc * 128:(cc + 1) * 128])
            nc.sync.dma_start(out=wk_sb, in_=wk_v[:, :, cc * 128:(cc + 1) * 128])
            qt_sb = qk_pool.tile([P, S], BF16, tag="qt")
            kt_sb = qk_pool.tile([P, S], BF16, tag="kt")
            for tt in range(TT):
                xt_t = xt_pool.tile([P, DIN_C, 512], BF16, tag="xt")
                nc.sync.dma_start(out=xt_t[:, 0:4, :],
                                  in_=xt_v[:, 0:4, tt * 512:(tt + 1) * 512])
                nc.sync.dma_start(out=xt_t[:, 4:8, :],
                                  in_=xt_v[:, 4:8, tt * 512:(tt + 1) * 512])
                qps = proj_ps.tile([P, 512], F32, tag="pp", name=f"qps_{cc}_{tt}")
                for kc in range(DIN_C):
                    nc.tensor.matmul(
                        qps, wq_sb[:, kc, :], xt_t[:, kc, :],
                        start=(kc == 0), stop=(kc == DIN_C - 1),
                    )
                nc.vector.tensor_copy(qt_sb[:, tt * 512:(tt + 1) * 512], qps)
                kps = proj_ps.tile([P, 512], F32, tag="pp", name=f"kps_{cc}_{tt}")
                for kc in range(DIN_C):
                    nc.tensor.matmul(
                        kps, wk_sb[:, kc, :], xt_t[:, kc, :],
                        start=(kc == 0), stop=(kc == DIN_C - 1),
                    )
                nc.vector.tensor_copy(kt_sb[:, tt * 512:(tt + 1) * 512], kps)

            for qt in range(QT):
                ops = [att_ps.tile([P, 512], F32, tag=f"ops{h2}", name=f"ops{h2}_{cc}_{qt}") for h2 in range(2)]
                ngrp = 2 * qt + 2
                last_kc = 2 * ngrp - 1
                for grp in range(ngrp):
                    sps = [att_ps.tile([P, 1024], F32, tag=f"sps{h2}", name=f"sps{h2}_{cc}_{qt}_{grp}") for h2 in range(2)]
                    for j in range(2):
                        kc = grp * 2 + j
                        for h2 in range(2):
                            b0 = h2 * 64
                            nc.tensor.matmul(
                                sps[h2][:, j * 512:(j + 1) * 512],
                                kt_sb[b0:b0 + 64, kc * 128:(kc + 1) * 128],
                                qt_sb[b0:b0 + 64, qt * 512:(qt + 1) * 512],
                                start=True,
                                stop=True,
                            )
                    pts = []
                    for h2 in range(2):
                        p_t = p_pool.tile([P, 1024], BF16, tag=f"p{h2}", name=f"p{h2}_{cc}_{qt}_{grp}")
                        nc.scalar.activation(p_t, sps[h2], EXP, scale=0.125)
                        pts.append(p_t)
                    if grp >= 2 * qt:  # diagonal groups: causal mask
                        r0 = (grp - 2 * qt) * 2
                        for h2 in range(2):
                            for j in range(2):
                                nc.vector.tensor_mul(
                                    pts[h2][:, j * 512:(j + 1) * 512],
                                    pts[h2][:, j * 512:(j + 1) * 512],
                                    mask_sb[:, r0 + j, :],
                                )
                    for j in range(2):
                        kc = grp * 2 + j
                        for h2 in range(2):
                            nc.tensor.matmul(
                                ops[h2][0:VH, :],
                                v_sb[:, kc, 2 * cc + h2, 0:VH],
                                pts[h2][:, j * 512:(j + 1) * 512],
                                start=(kc == 0),
                                stop=(kc == last_kc),
                            )
                for h2 in range(2):
                    # Move U out of PSUM immediately (frees the opsum bank
                    # for the next query tile), compute 1/s = exp(-ln s) on
                    # ScalarE (DVE's iterative divide is ~8 cyc/elem on one
                    # lane), broadcast it across 64 partitions via a K=1
                    # outer product on PE, then scale U.
                    u_sb = misc.tile([64, 512], F32, tag=f"u{h2}",
                                     name=f"u{h2}_{cc}_{qt}")
                    nc.vector.tensor_copy(u_sb, ops[h2][0:64, :])
                    rrow = misc.tile([P, 512], F32, tag="rrow",
                                     name=f"rrow{h2}_{cc}_{qt}")
                    nc.scalar.activation(rrow[64:65, :], ops[h2][64:65, :],
                                         mybir.ActivationFunctionType.Ln)
                    rexp = misc.tile([P, 512], F32, tag="rexp",
                                     name=f"rexp{h2}_{cc}_{qt}")
                    nc.scalar.activation(rexp[64:65, :], rrow[64:65, :],
                                         EXP, scale=-1.0)
                    rec = misc.tile([64, 512], F32, tag="rec",
                                    name=f"rec{h2}_{cc}_{qt}")
                    rdram = dram_pool.tile([1, 512], F32, tag="rd",
                                           name=f"rd{h2}_{cc}_{qt}")
                    nc.sync.dma_start(out=rdram, in_=rexp[64:65, :])
                    rsrc = rdram[0:1, :]
                    nc.sync.dma_start(
                        out=rec,
                        in_=bass.AP(tensor=rsrc.tensor, offset=rsrc.offset,
                                    ap=[[0, 64], [1, 512]]),
                    )
                    nc.vector.tensor_mul(
                        ot_sb[h2 * 64:h2 * 64 + 64, cc, qt * 512:(qt + 1) * 512],
                        u_sb,
                        rec,
                    )

        # ---- output projection (partial, host sums across head groups) ----
        for tt in range(TT):
            for oc in range(D // P):
                ops_o = proj_ps.tile([P, 512], F32, tag="pp")
                for c2 in range(CC):
                    nc.tensor.matmul(
                        ops_o,
                        wot_sb[:, c2, oc * 128:(oc + 1) * 128],
                        ot_sb[:, c2, tt * 512:(tt + 1) * 512],
                        start=(c2 == 0),
                        stop=(c2 == CC - 1),
                    )
                y_t = yt_pool.tile([P, 512], F32, tag="yt")
                nc.vector.tensor_copy(y_t, ops_o)
                nc.sync.dma_start(
                    out=yt_v[oc * 128:(oc + 1) * 128, tt * 512:(tt + 1) * 512],
                    in_=y_t,
                )

    nc.finalize()
    return nc


def _make_mask():
    keys = np.arange(4)[None, :, None] * 128 + np.arange(128)[:, None, None]
    qs = np.arange(512)[None, None, :]
    return (keys <= qs).astype(np.float32)


def kernel(X, W_Q, W_K, W_V, W_O):
    global LAST_RESULT, _NC_CACHE
    X = np.asarray(X, dtype=np.float32)
    W_Q = np.asarray(W_Q, dtype=np.float32)
    W_K = np.asarray(W_K, dtype=np.float32)
    W_V = np.asarray(W_V, dtype=np.float32)
    W_O = np.asarray(W_O, dtype=np.float32)

    mask = _make_mask().astype(ml_dtypes.bfloat16)
    in_maps = []
    for c in range(8):
        b, g = c // 2, c % 2
        sl = slice(g * 512, (g + 1) * 512)
        in_maps.append({
            "ones": np.ones((128, 16 * 8 * 65), dtype=ml_dtypes.bfloat16),
            "xt": np.ascontiguousarray(X[b].T).astype(ml_dtypes.bfloat16),
            "wqt": np.ascontiguousarray(W_Q[sl, :].T).astype(ml_dtypes.bfloat16),
            "wkt": np.ascontiguousarray(W_K[sl, :].T).astype(ml_dtypes.bfloat16),
            "wvt": np.ascontiguousarray(W_V[sl, :].T).astype(ml_dtypes.bfloat16),
            "wot": np.ascontiguousarray(W_O[:, sl].T).astype(ml_dtypes.bfloat16),
            "mask": mask,
        })

    if _NC_CACHE is None:
        _NC_CACHE = build_nc()
    res = run_bass_kernel_spmd(_NC_CACHE, in_maps, core_ids=list(range(8)))
    LAST_RESULT = res

    out = np.empty((B, S, D), dtype=np.float32)
    for b in range(B):
        yt = res.results[2 * b]["yt"] + res.results[2 * b + 1]["yt"]
        out[b] = yt.T
    return out


# revision 17
# speedup vs baseline: 1.5382x; 1.0189x over previous
"""Causal multi-head self-attention on 8 Trainium2 NeuronCores.

Problem: X[4, 2048, 1024] fp32, W_Q/W_K/W_V/W_O [1024, 1024] fp32,
16 heads x 64 dims, causal softmax attention + output projection.

Sharding: core c handles batch b = c//2 and head-group g = c%2
(heads g*8..g*8+8, i.e. 512 of the 1024 channels).  Each core computes
its 8 heads' Q/K/V projections, causal attention, and a partial output
projection against W_O[:, g*512:(g+1)*512]; the host sums the two
partial outputs per batch (the "all-reduce after W_O" step).

Device kernel layout notes:
 - All matmuls use float32r (fp32 bits, reduced-precision multiply) which
   streams at 1 cycle/row when the moving free dim >= 256, vs 4 cycles
   for full fp32.
 - Q/K are produced transposed ([channels, tokens]) so attention scores
   are computed as S_T[keys, q] = K_slice^T-free matmuls with Dh=64 on
   the contraction partitions; two heads (partition bases 0/64) run as
   row-tiled concurrent matmuls.
 - Softmax skips the max-subtraction (scores are bounded ~|1.9| after
   the 1/8 scale, applied via the activation's free affine).  exp runs
   on ScalarE over [128, 1024] PSUM groups.
 - Causal masking multiplies the diagonal score blocks by a 0/1 mask
   after exp (mask depends only on the key/query offset within the
   block, so one [128, 4, 512] constant serves every query tile).
 - V is stored [tokens, 512 ch + 64 ones]; using [V_head | ones] as the
   stationary operand of the P*V matmul makes PSUM rows 0..63 the
   unnormalized output and rows 64..127 the softmax row-sums, so the
   normalization is a reciprocal + multiply with no cross-partition
   reduction.
"""

import sys

if "/opt/trn_rl_repo" not in sys.path:
    sys.path.insert(0, "/opt/trn_rl_repo")

from contextlib import ExitStack

import ml_dtypes
import numpy as np

import concourse.bacc as bacc
import concourse.bass as bass
import concourse.hw_specs as _hw_specs
import concourse.tile as tile
from concourse import mybir
from concourse.bass_utils import run_bass_kernel_spmd

# Bias the activation-table chooser so Exp resolves to the set that also
# contains Ln ("natural_log_exp_and_others"): the kernel interleaves Exp
# (softmax) with Ln (reciprocal via exp(-ln s)), and per-function minimal
# sets would thrash the ~2.7us ACT table load on every switch.
_orig_get_activation_tables = _hw_specs.get_activation_tables


def _patched_activation_tables(arch):
    exp_fn = mybir.ActivationFunctionType.Exp
    out = {}
    for name, fns in _orig_get_activation_tables(arch).items():
        if name != "natural_log_exp_and_others" and exp_fn in fns:
            fns = [f for f in fns if f != exp_fn]
        out[name] = set(fns)
    return out


bacc.get_activation_tables = _patched_activation_tables

B = 4
S = 2048
D = 1024
H = 16
DH = 64

P = 128
DIN_C = D // P        # 8 contraction chunks for the projections
CC = 4                # channel chunks per core (512 / 128)
NHEAD = 8             # heads per core
QT = S // 512         # query tiles of 512
TT = S // 512         # token tiles of 512
VH = 65               # per-head V block: 64 dims + 1 ones column

F32R = mybir.dt.float32r
F32 = mybir.dt.float32
BF16 = mybir.dt.bfloat16

LAST_RESULT = None
_NC_CACHE = None


def build_nc():
    nc = bacc.Bacc()

    xt_d = nc.dram_tensor("xt", [D, S], BF16, kind="ExternalInput")
    wqt_d = nc.dram_tensor("wqt", [D, 512], BF16, kind="ExternalInput")
    wkt_d = nc.dram_tensor("wkt", [D, 512], BF16, kind="ExternalInput")
    wvt_d = nc.dram_tensor("wvt", [D, 512], BF16, kind="ExternalInput")
    wot_d = nc.dram_tensor("wot", [512, D], BF16, kind="ExternalInput")
    mask_d = nc.dram_tensor("mask", [P, 4, 512], BF16, kind="ExternalInput")
    ones_d = nc.dram_tensor("ones", [P, (S // P) * NHEAD * VH], BF16, kind="ExternalInput")
    yt_d = nc.dram_tensor("yt", [D, S], F32, kind="ExternalOutput")

    xt_v = xt_d[:, :].rearrange("(kc p) t -> p kc t", p=P)
    wq_v = wqt_d[:, :].rearrange("(kc p) c -> p kc c", p=P)
    wk_v = wkt_d[:, :].rearrange("(kc p) c -> p kc c", p=P)
    wv_v = wvt_d[:, :].rearrange("(kc p) c -> p kc c", p=P)
    wot_v = wot_d[:, :].rearrange("(cc p) o -> p cc o", p=P)
    yt_v = yt_d[:, :]

    EXP = mybir.ActivationFunctionType.Exp

    with tile.TileContext(nc) as tc, ExitStack() as ctx:
        singles = ctx.enter_context(tc.tile_pool(name="singles", bufs=1))
        xt_pool = ctx.enter_context(tc.tile_pool(name="xtp", bufs=2))
        qk_pool = ctx.enter_context(tc.tile_pool(name="qkp", bufs=2))
        w_pool = ctx.enter_context(tc.tile_pool(name="wp", bufs=1))
        p_pool = ctx.enter_context(tc.tile_pool(name="pp", bufs=2))
        misc = ctx.enter_context(tc.tile_pool(name="misc", bufs=2))
        yt_pool = ctx.enter_context(tc.tile_pool(name="ytp", bufs=2))
        proj_ps = ctx.enter_context(tc.tile_pool(name="proj_ps", bufs=2, space="PSUM"))
        dram_pool = ctx.enter_context(tc.tile_pool(name="drp", bufs=2, space="DRAM"))
        att_ps = ctx.enter_context(tc.tile_pool(name="att_ps", bufs=1, space="PSUM"))

        v_sb = singles.tile([P, S // P, NHEAD, VH], BF16)
        ot_sb = singles.tile([P, CC, S], BF16)
        wot_sb = singles.tile([P, CC, D], BF16)
        mask_sb = singles.tile([P, 4, 512], BF16)
        # fill v_sb with 1.0 wholesale; V-projection copies overwrite the
        # data columns, leaving col 64 of each head block as the ones col.
        nc.sync.dma_start(out=v_sb[:, :, :, :], in_=ones_d[:, :])
        nc.sync.dma_start(out=mask_sb, in_=mask_d[:, :, :])
        nc.sync.dma_start(out=wot_sb, in_=wot_v)

        # V-projection weights (consumed during the cc==0 pass below)
        wv_sb = w_pool.tile([P, DIN_C, 512], BF16, tag="wv")
        nc.sync.dma_start(out=wv_sb[:, 0:2, :], in_=wv_v[:, 0:2, :])
        nc.sync.dma_start(out=wv_sb[:, 2:8, :], in_=wv_v[:, 2:8, :])

        # ---- per channel-chunk: Q/K projection + attention for 2 heads ----
        for cc in range(CC):
            wq_sb = w_pool.tile([P, DIN_C, 128], BF16, tag="wq")
            wk_sb = w_pool.tile([P, DIN_C, 128], BF16, tag="wk")
            nc.sync.dma_start(out=wq_sb, in_=wq_v[:, :, cc * 128:(cc + 1) * 128])
            nc.sync.dma_start(out=wk_sb, in_=wk_v[:, :, cc * 128:(cc + 1) * 128])
            qt_sb = qk_pool.tile([P, S], BF16, tag="qt")
            kt_sb = qk_pool.tile([P, S], BF16, tag="kt")
            for tt in range(TT):
                xt_t = xt_pool.tile([P, DIN_C, 512], BF16, tag="xt")
                nc.sync.dma_start(out=xt_t[:, 0:4, :],
                                  in_=xt_v[:, 0:4, tt * 512:(tt + 1) * 512])
                nc.sync.dma_start(out=xt_t[:, 4:8, :],
                                  in_=xt_v[:, 4:8, tt * 512:(tt + 1) * 512])
                if cc == 0:
                    # V projection interleaved with cc0 Q/K so attention
                    # (and ScalarE) starts as soon as token-tile 0 is done.
                    for sub in range(4):
                        vps = proj_ps.tile([P, 512], F32, tag="pp",
                                           name=f"vps_{tt}_{sub}")
                        for kc in range(DIN_C):
                            nc.tensor.matmul(
                                vps,
                                xt_t[:, kc, sub * 128:(sub + 1) * 128],
                                wv_sb[:, kc, :],
                                start=(kc == 0),
                                stop=(kc == DIN_C - 1),
                            )
                        nc.vector.tensor_copy(v_sb[:, tt * 4 + sub, :, 0:64], vps)
                qps = proj_ps.tile([P, 512], F32, tag="pp", name=f"qps_{cc}_{tt}")
                for kc in range(DIN_C):
                    nc.tensor.matmul(
                        qps, wq_sb[:, kc, :], xt_t[:, kc, :],
                        start=(kc == 0), stop=(kc == DIN_C - 1),
                    )
                nc.vector.tensor_copy(qt_sb[:, tt * 512:(tt + 1) * 512], qps)
                kps = proj_ps.tile([P, 512], F32, tag="pp", name=f"kps_{cc}_{tt}")
                for kc in range(DIN_C):
                    nc.tensor.matmul(
                        kps, wk_sb[:, kc, :], xt_t[:, kc, :],
                        start=(kc == 0), stop=(kc == DIN_C - 1),
                    )
                nc.vector.tensor_copy(kt_sb[:, tt * 512:(tt + 1) * 512], kps)

            for qt in range(QT):
                ops = [att_ps.tile([P, 512], F32, tag=f"ops{h2}", name=f"ops{h2}_{cc}_{qt}") for h2 in range(2)]
                ngrp = 2 * qt + 2
                last_kc = 2 * ngrp - 1
                for grp in range(ngrp):
                    sps = [att_ps.tile([P, 1024], F32, tag=f"sps{h2}", name=f"sps{h2}_{cc}_{qt}_{grp}") for h2 in range(2)]
                    for j in range(2):
                        kc = grp * 2 + j
                        for h2 in range(2):
                            b0 = h2 * 64
                            nc.tensor.matmul(
                                sps[h2][:, j * 512:(j + 1) * 512],
                                kt_sb[b0:b0 + 64, kc * 128:(kc + 1) * 128],
                                qt_sb[b0:b0 + 64, qt * 512:(qt + 1) * 512],
                                start=True,
                                stop=True,
                            )
                    pts = []
                    for h2 in range(2):
                        p_t = p_pool.tile([P, 1024], BF16, tag=f"p{h2}", name=f"p{h2}_{cc}_{qt}_{grp}")
                        nc.scalar.activation(p_t, sps[h2], EXP, scale=0.125)
                        pts.append(p_t)
                    if grp >= 2 * qt:  # diagonal groups: causal mask
                        r0 = (grp - 2 * qt) * 2
                        for h2 in range(2):
                            for j in range(2):
                                nc.vector.tensor_mul(
                                    pts[h2][:, j * 512:(j + 1) * 512],
                                    pts[h2][:, j * 512:(j + 1) * 512],
                                    mask_sb[:, r0 + j, :],
                                )
                    for j in range(2):
                        kc = grp * 2 + j
                        for h2 in range(2):
                            nc.tensor.matmul(
                                ops[h2][0:VH, :],
                                v_sb[:, kc, 2 * cc + h2, 0:VH],
                                pts[h2][:, j * 512:(j + 1) * 512],
                                start=(kc == 0),
                                stop=(kc == last_kc),
                            )
                for h2 in range(2):
                    # Move U out of PSUM immediately (frees the opsum bank
                    # for the next query tile); bounce the softmax-sum row
                    # through DRAM to broadcast it across 64 partitions, and
                    # do the division on the otherwise-idle GpSimd engine.
                    u_sb = misc.tile([64, 512], F32, tag=f"u{h2}",
                                     name=f"u{h2}_{cc}_{qt}")
                    nc.vector.tensor_copy(u_sb, ops[h2][0:64, :])
                    rrow = misc.tile([P, 512], F32, tag="rrow",
                                     name=f"rrow{h2}_{cc}_{qt}")
                    nc.scalar.activation(rrow[64:65, :], ops[h2][64:65, :],
                                         mybir.ActivationFunctionType.Ln)
                    rexp = misc.tile([P, 512], F32, tag="rexp",
                                     name=f"rexp{h2}_{cc}_{qt}")
                    nc.scalar.activation(rexp[64:65, :], rrow[64:65, :],
                                         EXP, scale=-1.0)
                    rdram = dram_pool.tile([1, 512], F32, tag="rd",
                                           name=f"rd{h2}_{cc}_{qt}")
                    nc.sync.dma_start(out=rdram, in_=rexp[64:65, :])
                    rec = misc.tile([64, 512], F32, tag="rec",
                                    name=f"rec{h2}_{cc}_{qt}")
                    rsrc = rdram[0:1, :]
                    nc.sync.dma_start(
                        out=rec,
                        in_=bass.AP(tensor=rsrc.tensor, offset=rsrc.offset,
                                    ap=[[0, 64], [1, 512]]),
                    )
                    nc.vector.tensor_mul(
                        ot_sb[h2 * 64:h2 * 64 + 64, cc,
                              qt * 512:(qt + 1) * 512],
                        u_sb,
                        rec,
                    )

                if cc == CC - 1:
                    # Output projection for token-tile qt: all four channel
                    # chunks of ot are final once cc==3 finishes this qt,
                    # so the partial W_O product overlaps the remaining
                    # attention instead of running as a serial tail.
                    tt_o = qt
                    for oc in range(D // P):
                        ops_o = proj_ps.tile([P, 512], F32, tag="pp",
                                             name=f"ops_o_{tt_o}_{oc}")
                        for c2 in range(CC):
                            nc.tensor.matmul(
                                ops_o,
                                wot_sb[:, c2, oc * 128:(oc + 1) * 128],
                                ot_sb[:, c2, tt_o * 512:(tt_o + 1) * 512],
                                start=(c2 == 0),
                                stop=(c2 == CC - 1),
                            )
                        y_t = yt_pool.tile([P, 512], F32, tag="yt",
                                           name=f"yt_{tt_o}_{oc}")
                        nc.vector.tensor_copy(y_t, ops_o)
                        nc.sync.dma_start(
                            out=yt_v[oc * 128:(oc + 1) * 128,
                                     tt_o * 512:(tt_o + 1) * 512],
                            in_=y_t,
                        )



    nc.finalize()
    return nc


def _make_mask():
    keys = np.arange(4)[None, :, None] * 128 + np.arange(128)[:, None, None]
    qs = np.arange(512)[None, None, :]
    return (keys <= qs).astype(np.float32)


def kernel(X, W_Q, W_K, W_V, W_O):
    global LAST_RESULT, _NC_CACHE
    X = np.asarray(X, dtype=np.float32)
    W_Q = np.asarray(W_Q, dtype=np.float32)
    W_K = np.asarray(W_K, dtype=np.float32)
    W_V = np.asarray(W_V, dtype=np.float32)
    W_O = np.asarray(W_O, dtype=np.float32)

    mask = _make_mask().astype(ml_dtypes.bfloat16)
    in_maps = []
    for c in range(8):
        b, g = c // 2, c % 2
        sl = slice(g * 512, (g + 1) * 512)
        in_maps.append({
            "ones": np.ones((128, 16 * 8 * 65), dtype=ml_dtypes.bfloat16),
            "xt": np.ascontiguousarray(X[b].T).astype(ml_dtypes.bfloat16),
            "wqt": np.ascontiguousarray(W_Q[sl, :].T).astype(ml_dtypes.bfloat16),
            "wkt": np.ascontiguousarray(W_K[sl, :].T).astype(ml_dtypes.bfloat16),
            "wvt": np.ascontiguousarray(W_V[sl, :].T).astype(ml_dtypes.bfloat16),
            "wot": np.ascontiguousarray(W_O[:, sl].T).astype(ml_dtypes.bfloat16),
            "mask": mask,
        })

    if _NC_CACHE is None:
        _NC_CACHE = build_nc()
    res = run_bass_kernel_spmd(_NC_CACHE, in_maps, core_ids=list(range(8)))
    LAST_RESULT = res

    out = np.empty((B, S, D), dtype=np.float32)
    for b in range(B):
        yt = res.results[2 * b]["yt"] + res.results[2 * b + 1]["yt"]
        out[b] = yt.T
    return out
